# revision 13
# baseline (speedup 1.0000x reference)
"""DGMF loss kernel for Trainium2 (8 NeuronCores, data-parallel over N).

Math (per view v, per node n):
  S_k = sigma_k + 1e-6 I ;  A_k = S_k^{-1} = L_k L_k^T  (Cholesky of A)
  maha_nk = z_n^T A_k z_n - 2 (A_k mu_k) . z_n + mu_k^T A_k mu_k
  log_prob_nk = -0.5 maha_nk + C_k ;  energy_n = -logsumexp_k(log_prob)
  weights = softmax over views of -energy / TAU.

Device work, per 128-node chunk (fp32r = TF32-rate matmuls):
  Y[n, k*128+d'] = (z @ L_k)[n, d'] and bz[n, k] = z . b_k via one PE
  pass against a [128, 1032] stacked parameter matrix; then
  q_k(n) = sum_d' Y^2 with ACT Square+accum (k=0..3) and DVE bn_stats
  (k=4..7). A batched tail does logsumexp over k and the softmax over
  views. Host does the tiny param factorizations, z transpose/shard,
  unshard, and the scalar totals/penalty.
"""

import os

import numpy as np

import concourse.bass as bass
import concourse.mybir as mybir
from concourse import tile
from concourse.bass_utils import run_bass_kernel_spmd

V, N, K, D = 2, 100000, 8, 128
TAU = 1.0
LOG_2PI = float(np.log(2.0 * np.pi))

NCORES = 8
NPER = N // NCORES          # 12500 nodes per core
BIGF = 512                  # free-dim of one DMA tile of z^T
NPAD = 12800                # NPER padded to a multiple of BIGF
NBIG = NPAD // BIGF         # 25 DMA tiles per view
CPB = BIGF // 128           # 4 PE chunks per DMA tile
NCHUNK = NBIG * CPB         # 100 chunks per view
NCV = NCHUNK * V            # 200 chunk-views per core
WCOLS = K * D + K           # 1032 stacked param columns
BZG = 64                    # chunk-views batched per bz PSUM bank

LAST_RESULTS = None         # test harness reads exec_time_ns from here

f32 = mybir.dt.float32
f32r = mybir.dt.float32r
bf16 = mybir.dt.bfloat16
AF = mybir.ActivationFunctionType
ALU = mybir.AluOpType


def _split_multi_waits(nc):
    """This toolchain's walrus accepts at most one sem-wait per TPB
    instruction. Hoist extra waits onto EventSemaphore prefixes."""
    for fn in nc.m.functions:
        for blk in fn.blocks:
            new_insts = []
            for inst in blk.instructions:
                si = getattr(inst, "sync_info", None)
                if si is not None and len(si.on_wait) > 1:
                    for w in si.on_wait[1:]:
                        new_insts.append(
                            mybir.InstEventSemaphore(
                                name=nc.get_next_instruction_name(),
                                sync_info=mybir.SyncInfo(on_wait=[w], on_update=[]),
                                bass_nofuse=True,
                                engine=inst.engine,
                                ins=[],
                                outs=[],
                            )
                        )
                    si.on_wait = si.on_wait[:1]
                new_insts.append(inst)
            blk.instructions[:] = new_insts


def _flag(name, default="1"):
    return os.environ.get(name, default) == "1"


def _build_program():
    DO_ACT = _flag("K_ACT")
    DO_DVE = _flag("K_DVE")
    DO_TAIL = _flag("K_TAIL")
    N_PASS = int(os.environ.get("K_PASS", "3"))
    nc = bass.Bass(trn_type="TRN2", target_bir_lowering=False, debug=False)

    zTh = nc.dram_tensor("zTh", [V, D, NPAD], f32r, kind="ExternalInput")
    zTl = nc.dram_tensor("zTl", [V, D, NPAD], f32r, kind="ExternalInput")
    Wh = nc.dram_tensor("Wh", [V, D, WCOLS], f32r, kind="ExternalInput")
    Wl = nc.dram_tensor("Wl", [V, D, WCOLS], f32r, kind="ExternalInput")
    Ctab = nc.dram_tensor("Ctab", [D, V * K], f32, kind="ExternalInput")
    eout = nc.dram_tensor("eout", [D, NCV], f32, kind="ExternalOutput")
    wout = nc.dram_tensor("wout", [D, NCV], f32, kind="ExternalOutput")

    with tile.TileContext(nc) as tc:
        with (
            tc.tile_pool(name="params", bufs=1) as params,
            tc.tile_pool(name="stage", bufs=1) as stage,
            tc.tile_pool(name="zin", bufs=4) as zin,
            tc.tile_pool(name="scratch", bufs=1) as scratch,
            tc.tile_pool(name="ps", bufs=3, space="PSUM") as ps,
            tc.tile_pool(name="psbz", bufs=1, space="PSUM") as psbz,
            tc.tile_pool(name="psscr", bufs=1, space="PSUM") as psscr,
        ):
            # --- persistent params ---
            wh_t, wl_t = [], []
            for v in range(V):
                wt = params.tile([D, WCOLS], f32r, tag=f"wh{v}")
                nc.sync.dma_start(wt[:], zTv(Wh, v))
                wh_t.append(wt)
                wt2 = params.tile([D, WCOLS], f32r, tag=f"wl{v}")
                nc.sync.dma_start(wt2[:], zTv(Wl, v))
                wl_t.append(wt2)
            c_t = params.tile([D, V * K], f32)
            nc.sync.dma_start(c_t[:], Ctab.ap())

            # --- persistent staging ---
            q_buf = stage.tile([D, NCV * K], f32, tag="q")
            bz_buf = stage.tile([D, NCV * K], f32, tag="bz")
            st_buf = stage.tile([D, NCV * 18], f32, tag="st")
            sq_scr = psscr.tile([D, D], f32, tag="sqscr")

            # --- main loop ---
            for big in range(NBIG):
                for v in range(V):
                    zth = zin.tile([D, BIGF], f32r, tag="zth")
                    nc.sync.dma_start(
                        zth[:], zTv(zTh, v)[:, big * BIGF:(big + 1) * BIGF]
                    )
                    ztl = zin.tile([D, BIGF], f32r, tag="ztl")
                    nc.sync.dma_start(
                        ztl[:], zTv(zTl, v)[:, big * BIGF:(big + 1) * BIGF]
                    )
                    zh_bc = zth[:].bitcast(bf16)
                    zl_bc = ztl[:].bitcast(bf16)
                    for c in range(CPB):
                        cv = (big * CPB + c) * V + v
                        g = cv % BZG
                        if g == 0:
                            bz_ps = psbz.tile([D, BZG * K], f32, tag="bzps")
                        y = ps.tile([D, K * D], f32, tag="y")
                        # wait-consolidator: bf16 matmuls (normal multi-wait
                        # path) touch the DMA'd tiles and the PSUM slots so
                        # the fp32r matmuls below need no waits (walrus's
                        # fused fp32r weight-load takes at most one).
                        nc.tensor.matmul(
                            y[:1, :1], zh_bc[:, :1], zl_bc[:, :1],
                            start=True, stop=True,
                        )
                        if g == 0:
                            nc.tensor.matmul(
                                bz_ps[:1, :1], zh_bc[:, :1], zl_bc[:, :1],
                                start=True, stop=True,
                            )
                        # Y = z_h W_h + z_h W_l + z_l W_h  (PSUM accumulate)
                        lh = zth[:, c * 128:(c + 1) * 128]
                        ll = ztl[:, c * 128:(c + 1) * 128]
                        passes = [
                            (lh, wh_t[v], True, False),
                            (lh, wl_t[v], False, False),
                            (ll, wh_t[v], False, True),
                        ][:N_PASS]
                        passes[-1] = (passes[-1][0], passes[-1][1],
                                      passes[-1][2], True)
                        for (lhsT, wt, first, last) in passes:
                            nc.tensor.matmul(
                                y[:, 0:512], lhsT, wt[:, 0:512],
                                start=first, stop=last,
                            )
                            nc.tensor.matmul(
                                y[:, 512:1024], lhsT, wt[:, 512:1024],
                                start=first, stop=last,
                            )
                            nc.tensor.matmul(
                                bz_ps[:, g * K:(g + 1) * K],
                                lhsT, wt[:, 1024:1032],
                                start=first, stop=last,
                            )
                        # ACT: k = 0..2 square+accum
                        for k in range(3 if DO_ACT else 0):
                            nc.scalar.activation(
                                sq_scr[:],
                                y[:, k * 128:(k + 1) * 128],
                                AF.Square,
                                accum_out=q_buf[:, cv * K + k:cv * K + k + 1],
                            )
                        # DVE: k = 3..7 via bn_stats (pairs use the FD/2
                        # half-split so one op yields two per-k stats)
                        if DO_DVE:
                            st0 = cv * 18
                            # bn_stats "halves" are even/odd element
                            # index: interleave the two k-blocks (inner AP
                            # dim = k, stride 128) so evens = first k, odds
                            # = second. Direct emission: the bass wrapper
                            # rejects 3D inputs, walrus accepts them.
                            for pair, base in ((0, 3), (1, 5)):
                                in3d = y[:, base * 128:(base + 2) * 128]
                                in3d = in3d.rearrange("p (k d) -> p d k", k=2)
                                nc.vector.add_instruction(
                                    mybir.InstBNStats(
                                        name=nc.get_next_instruction_name(),
                                        ins=[nc.vector.lower_ap(in3d)],
                                        outs=[nc.vector.lower_ap(
                                            st_buf[:, st0 + 6 * pair:
                                                   st0 + 6 * pair + 6])],
                                    )
                                )
                            nc.vector.bn_stats(
                                st_buf[:, st0 + 12:st0 + 18],
                                y[:, 7 * 128:8 * 128],
                            )
                        if DO_ACT and (g == BZG - 1 or cv == NCV - 1):
                            nc.scalar.activation(
                                bz_buf[:, (cv - g) * K:(cv + 1) * K],
                                bz_ps[:, :(g + 1) * K],
                                AF.Copy,
                            )

            # --- tail: bn_stats -> q for k=3..7 ---
            st = st_buf[:]
            def sfield(slot, f):
                return st[:, slot * 6 + f::18]          # [128, NCV]
            ta = stage.tile([D, NCV], f32, tag="ta")
            qv = q_buf[:]
            # pair slots 0,1: halves have count 128 -> q = M2 + 128*mean^2
            for slot, ks in ((0, (3, 4)), (1, (5, 6))):
                for half, k in enumerate(ks):
                    mn_f = sfield(slot, 1 + 3 * half)
                    m2_f = sfield(slot, 2 + 3 * half)
                    nc.vector.tensor_tensor(ta[:], mn_f, mn_f, ALU.mult)
                    nc.vector.scalar_tensor_tensor(
                        qv[:, k::K], ta[:], 128.0, m2_f, ALU.mult, ALU.add,
                    )
            # slot 2 (k=7, FD=128): halves of 64
            tb = stage.tile([D, NCV], f32, tag="tb")
            tc2 = stage.tile([D, NCV], f32, tag="tc2")
            nc.vector.tensor_tensor(ta[:], sfield(2, 1), sfield(2, 1), ALU.mult)
            nc.vector.tensor_tensor(tb[:], sfield(2, 4), sfield(2, 4), ALU.mult)
            nc.vector.tensor_tensor(tb[:], ta[:], tb[:], ALU.add)
            nc.vector.tensor_tensor(tc2[:], sfield(2, 2), sfield(2, 5), ALU.add)
            nc.vector.scalar_tensor_tensor(
                qv[:, 7::K], tb[:], 64.0, tc2[:], ALU.mult, ALU.add,
            )
            # lp = -0.5*q + bz + C
            lp = stage.tile([D, NCV * K], f32, tag="lp")
            nc.vector.scalar_tensor_tensor(
                lp[:], q_buf[:], -0.5, bz_buf[:], ALU.mult, ALU.add,
            )
            lp3 = lp[:].rearrange("p (ch vk) -> p ch vk", vk=V * K)
            cb = c_t[:].unsqueeze(1).broadcast_to([D, NCHUNK, V * K])
            nc.vector.tensor_tensor(lp3, lp3, cb, ALU.add)
            # logsumexp over k
            mx = stage.tile([D, NCV], f32, tag="mx")
            lpk = lp[:].rearrange("p (cv k) -> p cv k", k=K)
            nc.vector.tensor_reduce(
                mx[:], lpk, mybir.AxisListType.X, ALU.max,
            )
            eb = stage.tile([D, NCV * K], f32, tag="eb")
            mxb = mx[:].unsqueeze(2).broadcast_to([D, NCV, K])
            nc.vector.scalar_tensor_tensor(
                eb[:].rearrange("p (cv k) -> p cv k", k=K),
                lpk, 1.0, mxb, ALU.mult, ALU.subtract,
            )
            nc.scalar.activation(eb[:], eb[:], AF.Exp)
            se = stage.tile([D, NCV], f32, tag="se")
            nc.vector.tensor_reduce(
                se[:], eb[:].rearrange("p (cv k) -> p cv k", k=K),
                mybir.AxisListType.X, ALU.add,
            )
            lgs = stage.tile([D, NCV], f32, tag="lgs")
            nc.scalar.activation(lgs[:], se[:], AF.Ln)
            ebuf = stage.tile([D, NCV], f32, tag="ebuf")
            nc.vector.scalar_tensor_tensor(
                ebuf[:], mx[:], -1.0, lgs[:], ALU.mult, ALU.subtract,
            )
            nc.sync.dma_start(eout.ap(), ebuf[:])
            # weights: softmax over views of -E
            mn = stage.tile([D, NCHUNK], f32, tag="mn")
            ev = ebuf[:].rearrange("p (ch v) -> p ch v", v=V)
            nc.vector.tensor_reduce(mn[:], ev, mybir.AxisListType.X, ALU.min)
            ub = stage.tile([D, NCV], f32, tag="ub")
            mnb = mn[:].unsqueeze(2).broadcast_to([D, NCHUNK, V])
            nc.vector.scalar_tensor_tensor(
                ub[:].rearrange("p (ch v) -> p ch v", v=V),
                ev, -1.0, mnb, ALU.mult, ALU.add,
            )
            nc.scalar.activation(ub[:], ub[:], AF.Exp)
            su = stage.tile([D, NCHUNK], f32, tag="su")
            nc.vector.tensor_reduce(
                su[:], ub[:].rearrange("p (ch v) -> p ch v", v=V),
                mybir.AxisListType.X, ALU.add,
            )
            rcp = stage.tile([D, NCHUNK], f32, tag="rcp")
            nc.vector.reciprocal(rcp[:], su[:])
            wb = stage.tile([D, NCV], f32, tag="wb")
            rcb = rcp[:].unsqueeze(2).broadcast_to([D, NCHUNK, V])
            nc.vector.tensor_tensor(
                wb[:].rearrange("p (ch v) -> p ch v", v=V),
                ub[:].rearrange("p (ch v) -> p ch v", v=V),
                rcb, ALU.mult,
            )
            nc.sync.dma_start(wout.ap(), wb[:])

    _split_multi_waits(nc)
    return nc


def zTv(t, v):
    """Index a [V, ...] DRAM tensor ap down to view v."""
    return t.ap()[v]


def _tf32(x):
    """Round fp32 to fp32r (TF32: 10 explicit mantissa bits), RNE."""
    u = np.ascontiguousarray(x, dtype=np.float32).view(np.uint32)
    r = ((u >> 13) & np.uint32(1)).astype(np.uint32)
    u = (u + np.uint32(0x0FFF) + r) & np.uint32(0xFFFFE000)
    return u.view(np.float32)


def _host_params(phi, mu, sigma):
    """fp64 host-side factorization of the tiny GMM params."""
    S = sigma.astype(np.float64) + 1e-6 * np.eye(D)          # [V,K,D,D]
    Ls = np.linalg.cholesky(S)
    logdet = 2.0 * np.log(np.diagonal(Ls, axis1=-2, axis2=-1)).sum(-1)  # [V,K]
    A = np.linalg.inv(S)
    A = 0.5 * (A + A.transpose(0, 1, 3, 2))
    La = np.linalg.cholesky(A)                               # [V,K,D,D]
    b = np.einsum("vkde,vke->vkd", A, mu.astype(np.float64))  # [V,K,D]
    c = np.einsum("vkd,vkd->vk", mu.astype(np.float64), b)    # [V,K]
    ph = phi.astype(np.float64)
    logpi = ph - (np.max(ph, axis=1, keepdims=True)
                  + np.log(np.exp(ph - np.max(ph, axis=1, keepdims=True))
                           .sum(axis=1, keepdims=True)))
    C = -0.5 * c - 0.5 * D * LOG_2PI - 0.5 * logdet + logpi   # [V,K]

    Wstk = np.zeros((V, D, WCOLS), dtype=np.float64)
    for v in range(V):
        for k in range(K):
            Wstk[v, :, k * D:(k + 1) * D] = La[v, k]
            Wstk[v, :, K * D + k] = b[v, k]
    Wh = _tf32(Wstk.astype(np.float32))
    Wl = _tf32((Wstk - Wh.astype(np.float64)).astype(np.float32))
    Ctab = np.tile(C.reshape(1, V * K).astype(np.float32), (D, 1))

    # penalty is a pure function of sigma
    diag_S = np.diagonal(S, axis1=-2, axis2=-1)
    penalty = np.float32((1.0 / (diag_S + 1e-12)).sum())
    return Wh, Wl, Ctab, penalty


_NC_CACHE = None


def kernel(z, phi, mu, sigma):
    global LAST_RESULTS, _NC_CACHE
    z = np.asarray(z, dtype=np.float32)
    phi = np.asarray(phi, dtype=np.float32)
    mu = np.asarray(mu, dtype=np.float32)
    sigma = np.asarray(sigma, dtype=np.float32)

    Wh, Wl, Ctab, penalty = _host_params(phi, mu, sigma)

    if _NC_CACHE is None:
        _NC_CACHE = _build_program()
    nc = _NC_CACHE

    in_maps = []
    for core in range(NCORES):
        s = core * NPER
        zt = np.zeros((V, D, NPAD), dtype=np.float32)
        zt[:, :, :NPER] = z[:, s:s + NPER, :].transpose(0, 2, 1)
        zh = _tf32(zt)
        zl = _tf32(zt - zh)
        in_maps.append({"zTh": zh, "zTl": zl, "Wh": Wh, "Wl": Wl,
                        "Ctab": Ctab})

    res = run_bass_kernel_spmd(nc, in_maps, list(range(NCORES)))
    LAST_RESULTS = res

    energies = np.empty((N, V), dtype=np.float32)
    weights = np.empty((N, V), dtype=np.float32)
    for core in range(NCORES):
        s = core * NPER
        e = res.results[core]["eout"].reshape(D, NCHUNK, V)
        w = res.results[core]["wout"].reshape(D, NCHUNK, V)
        energies[s:s + NPER] = (
            e.transpose(1, 0, 2).reshape(NPAD, V)[:NPER]
        )
        weights[s:s + NPER] = (
            w.transpose(1, 0, 2).reshape(NPAD, V)[:NPER]
        )

    total_energies = np.float32(energies.astype(np.float64).sum())
    total_penalty = penalty
    return energies, weights, total_energies, total_penalty


# revision 14
# speedup vs baseline: 1.0054x; 1.0054x over previous
"""DGMF loss kernel for Trainium2 (8 NeuronCores, data-parallel over N).

Math (per view v, per node n):
  S_k = sigma_k + 1e-6 I ;  A_k = S_k^{-1} = L_k L_k^T  (Cholesky of A)
  maha_nk = z_n^T A_k z_n - 2 (A_k mu_k) . z_n + mu_k^T A_k mu_k
  log_prob_nk = -0.5 maha_nk + C_k ;  energy_n = -logsumexp_k(log_prob)
  weights = softmax over views of -energy / TAU.

Device work, per 128-node chunk (fp32r = TF32-rate matmuls):
  Y[n, k*128+d'] = (z @ L_k)[n, d'] and bz[n, k] = z . b_k via one PE
  pass against a [128, 1032] stacked parameter matrix; then
  q_k(n) = sum_d' Y^2 with ACT Square+accum (k=0..3) and DVE bn_stats
  (k=4..7). A batched tail does logsumexp over k and the softmax over
  views. Host does the tiny param factorizations, z transpose/shard,
  unshard, and the scalar totals/penalty.
"""

import os

import numpy as np

import concourse.bass as bass
import concourse.mybir as mybir
from concourse import tile
from concourse.bass_utils import run_bass_kernel_spmd

V, N, K, D = 2, 100000, 8, 128
TAU = 1.0
LOG_2PI = float(np.log(2.0 * np.pi))

NCORES = 8
NPER = N // NCORES          # 12500 nodes per core
BIGF = 512                  # free-dim of one DMA tile of z^T
NPAD = 12800                # NPER padded to a multiple of BIGF
NBIG = NPAD // BIGF         # 25 DMA tiles per view
CPB = BIGF // 128           # 4 PE chunks per DMA tile
NCHUNK = NBIG * CPB         # 100 chunks per view
NCV = NCHUNK * V            # 200 chunk-views per core
WCOLS = K * D + K           # 1032 stacked param columns
BZG = 64                    # chunk-views batched per bz PSUM bank

LAST_RESULTS = None         # test harness reads exec_time_ns from here

f32 = mybir.dt.float32
f32r = mybir.dt.float32r
bf16 = mybir.dt.bfloat16
AF = mybir.ActivationFunctionType
ALU = mybir.AluOpType


def _split_multi_waits(nc):
    """This toolchain's walrus accepts at most one sem-wait per TPB
    instruction. Hoist extra waits onto EventSemaphore prefixes."""
    for fn in nc.m.functions:
        for blk in fn.blocks:
            new_insts = []
            for inst in blk.instructions:
                si = getattr(inst, "sync_info", None)
                if si is not None and len(si.on_wait) > 1:
                    for w in si.on_wait[1:]:
                        new_insts.append(
                            mybir.InstEventSemaphore(
                                name=nc.get_next_instruction_name(),
                                sync_info=mybir.SyncInfo(on_wait=[w], on_update=[]),
                                bass_nofuse=True,
                                engine=inst.engine,
                                ins=[],
                                outs=[],
                            )
                        )
                    si.on_wait = si.on_wait[:1]
                new_insts.append(inst)
            blk.instructions[:] = new_insts


def _flag(name, default="1"):
    return os.environ.get(name, default) == "1"


def _build_program():
    DO_ACT = _flag("K_ACT")
    DO_DVE = _flag("K_DVE")
    DO_TAIL = _flag("K_TAIL")
    N_PASS = int(os.environ.get("K_PASS", "3"))
    nc = bass.Bass(trn_type="TRN2", target_bir_lowering=False, debug=False)

    zTh = nc.dram_tensor("zTh", [V, D, NPAD], f32r, kind="ExternalInput")
    zTl = nc.dram_tensor("zTl", [V, D, NPAD], f32r, kind="ExternalInput")
    Wh = nc.dram_tensor("Wh", [V, D, WCOLS], f32r, kind="ExternalInput")
    Wl = nc.dram_tensor("Wl", [V, D, WCOLS], f32r, kind="ExternalInput")
    Ctab = nc.dram_tensor("Ctab", [D, V * K], f32, kind="ExternalInput")
    eout = nc.dram_tensor("eout", [D, NCV], f32, kind="ExternalOutput")
    wout = nc.dram_tensor("wout", [D, NCV], f32, kind="ExternalOutput")

    with tile.TileContext(nc) as tc:
        with (
            tc.tile_pool(name="params", bufs=1) as params,
            tc.tile_pool(name="stage", bufs=1) as stage,
            tc.tile_pool(name="zin", bufs=4) as zin,
            tc.tile_pool(name="scratch", bufs=1) as scratch,
            tc.tile_pool(name="ps", bufs=3, space="PSUM") as ps,
            tc.tile_pool(name="psbz", bufs=1, space="PSUM") as psbz,
            tc.tile_pool(name="psscr", bufs=1, space="PSUM") as psscr,
        ):
            # --- persistent params ---
            wh_t, wl_t = [], []
            for v in range(V):
                wt = params.tile([D, WCOLS], f32r, tag=f"wh{v}")
                nc.sync.dma_start(wt[:], zTv(Wh, v))
                wh_t.append(wt)
                wt2 = params.tile([D, WCOLS], f32r, tag=f"wl{v}")
                nc.sync.dma_start(wt2[:], zTv(Wl, v))
                wl_t.append(wt2)
            c_t = params.tile([D, V * K], f32)
            nc.sync.dma_start(c_t[:], Ctab.ap())

            # --- persistent staging ---
            q_buf = stage.tile([D, NCV * K], f32, tag="q")
            bz_buf = stage.tile([D, NCV * K], f32, tag="bz")
            st_buf = stage.tile([D, NCV * 18], f32, tag="st")
            sq_scr = psscr.tile([D, D], f32, tag="sqscr")

            # --- main loop ---
            for big in range(NBIG):
                for v in range(V):
                    zth = zin.tile([D, BIGF], f32r, tag="zth")
                    nc.sync.dma_start(
                        zth[:], zTv(zTh, v)[:, big * BIGF:(big + 1) * BIGF]
                    )
                    ztl = zin.tile([D, BIGF], f32r, tag="ztl")
                    nc.sync.dma_start(
                        ztl[:], zTv(zTl, v)[:, big * BIGF:(big + 1) * BIGF]
                    )
                    zh_bc = zth[:].bitcast(bf16)
                    zl_bc = ztl[:].bitcast(bf16)
                    for c in range(CPB):
                        cv = (big * CPB + c) * V + v
                        g = cv % BZG
                        if g == 0:
                            bz_ps = psbz.tile([D, BZG * K], f32, tag="bzps")
                        y = ps.tile([D, K * D], f32, tag="y")
                        # wait-consolidator: bf16 matmuls (normal multi-wait
                        # path) touch the DMA'd tiles and the PSUM slots so
                        # the fp32r matmuls below need no waits (walrus's
                        # fused fp32r weight-load takes at most one).
                        nc.tensor.matmul(
                            y[:1, :1], zh_bc[:, :1], zl_bc[:, :1],
                            start=True, stop=True,
                        )
                        if g == 0:
                            nc.tensor.matmul(
                                bz_ps[:1, :1], zh_bc[:, :1], zl_bc[:, :1],
                                start=True, stop=True,
                            )
                        # Y = z_h W_h + z_h W_l + z_l W_h  (PSUM accumulate)
                        lh = zth[:, c * 128:(c + 1) * 128]
                        ll = ztl[:, c * 128:(c + 1) * 128]
                        passes = [
                            (lh, wh_t[v], True, False),
                            (lh, wl_t[v], False, False),
                            (ll, wh_t[v], False, True),
                        ][:N_PASS]
                        passes[-1] = (passes[-1][0], passes[-1][1],
                                      passes[-1][2], True)
                        for (lhsT, wt, first, last) in passes:
                            nc.tensor.matmul(
                                y[:, 0:512], lhsT, wt[:, 0:512],
                                start=first, stop=last,
                            )
                            nc.tensor.matmul(
                                y[:, 512:1024], lhsT, wt[:, 512:1024],
                                start=first, stop=last,
                            )
                            nc.tensor.matmul(
                                bz_ps[:, g * K:(g + 1) * K],
                                lhsT, wt[:, 1024:1032],
                                start=first, stop=last,
                            )
                        # ACT: k = 0..2 square+accum
                        for k in range(3 if DO_ACT else 0):
                            nc.scalar.activation(
                                sq_scr[:],
                                y[:, k * 128:(k + 1) * 128],
                                AF.Square,
                                accum_out=q_buf[:, cv * K + k:cv * K + k + 1],
                            )
                        # DVE: k = 3..7 via bn_stats (pairs use the FD/2
                        # half-split so one op yields two per-k stats)
                        if DO_DVE:
                            st0 = cv * 18
                            # bn_stats "halves" are even/odd element
                            # index: interleave the two k-blocks (inner AP
                            # dim = k, stride 128) so evens = first k, odds
                            # = second. Direct emission: the bass wrapper
                            # rejects 3D inputs, walrus accepts them.
                            for pair, base in ((0, 3), (1, 5)):
                                in3d = y[:, base * 128:(base + 2) * 128]
                                in3d = in3d.rearrange("p (k d) -> p d k", k=2)
                                nc.vector.add_instruction(
                                    mybir.InstBNStats(
                                        name=nc.get_next_instruction_name(),
                                        ins=[nc.vector.lower_ap(in3d)],
                                        outs=[nc.vector.lower_ap(
                                            st_buf[:, st0 + 6 * pair:
                                                   st0 + 6 * pair + 6])],
                                    )
                                )
                            nc.vector.bn_stats(
                                st_buf[:, st0 + 12:st0 + 18],
                                y[:, 7 * 128:8 * 128],
                            )
                        if DO_ACT and (g == BZG - 1 or cv == NCV - 1):
                            nc.scalar.activation(
                                bz_buf[:, (cv - g) * K:(cv + 1) * K],
                                bz_ps[:, :(g + 1) * K],
                                AF.Copy,
                            )

            # --- tail: bn_stats -> q for k=3..7 ---
            st = st_buf[:]
            def sfield(slot, f):
                return st[:, slot * 6 + f::18]          # [128, NCV]
            ta = stage.tile([D, NCV], f32, tag="ta")
            qv = q_buf[:]
            # pair slots 0,1: halves have count 128 -> q = M2 + 128*mean^2
            ta_k = {}
            for slot, ks in ((0, (3, 4)), (1, (5, 6))):
                for half, k in enumerate(ks):
                    mn_f = sfield(slot, 1 + 3 * half)
                    tk = stage.tile([D, NCV], f32, tag=f"ta{k}")
                    nc.scalar.activation(tk[:], mn_f, AF.Square)
                    ta_k[k] = tk
            for slot, ks in ((0, (3, 4)), (1, (5, 6))):
                for half, k in enumerate(ks):
                    m2_f = sfield(slot, 2 + 3 * half)
                    nc.vector.scalar_tensor_tensor(
                        qv[:, k::K], ta_k[k][:], 128.0, m2_f,
                        ALU.mult, ALU.add,
                    )
            # slot 2 (k=7, FD=128): halves of 64
            tb = stage.tile([D, NCV], f32, tag="tb")
            tc2 = stage.tile([D, NCV], f32, tag="tc2")
            nc.scalar.activation(ta[:], sfield(2, 1), AF.Square)
            nc.scalar.activation(tb[:], sfield(2, 4), AF.Square)
            nc.vector.tensor_tensor(tb[:], ta[:], tb[:], ALU.add)
            nc.vector.tensor_tensor(tc2[:], sfield(2, 2), sfield(2, 5), ALU.add)
            nc.vector.scalar_tensor_tensor(
                qv[:, 7::K], tb[:], 64.0, tc2[:], ALU.mult, ALU.add,
            )
            # lp = -0.5*q + bz + C
            lp = stage.tile([D, NCV * K], f32, tag="lp")
            nc.vector.scalar_tensor_tensor(
                lp[:], q_buf[:], -0.5, bz_buf[:], ALU.mult, ALU.add,
            )
            lp3 = lp[:].rearrange("p (ch vk) -> p ch vk", vk=V * K)
            cb = c_t[:].unsqueeze(1).broadcast_to([D, NCHUNK, V * K])
            nc.vector.tensor_tensor(lp3, lp3, cb, ALU.add)
            # logsumexp over k
            mx = stage.tile([D, NCV], f32, tag="mx")
            lpk = lp[:].rearrange("p (cv k) -> p cv k", k=K)
            nc.vector.tensor_reduce(
                mx[:], lpk, mybir.AxisListType.X, ALU.max,
            )
            eb = stage.tile([D, NCV * K], f32, tag="eb")
            mxb = mx[:].unsqueeze(2).broadcast_to([D, NCV, K])
            nc.vector.scalar_tensor_tensor(
                eb[:].rearrange("p (cv k) -> p cv k", k=K),
                lpk, 1.0, mxb, ALU.mult, ALU.subtract,
            )
            nc.scalar.activation(eb[:], eb[:], AF.Exp)
            se = stage.tile([D, NCV], f32, tag="se")
            nc.vector.tensor_reduce(
                se[:], eb[:].rearrange("p (cv k) -> p cv k", k=K),
                mybir.AxisListType.X, ALU.add,
            )
            lgs = stage.tile([D, NCV], f32, tag="lgs")
            nc.scalar.activation(lgs[:], se[:], AF.Ln)
            ebuf = stage.tile([D, NCV], f32, tag="ebuf")
            nc.vector.scalar_tensor_tensor(
                ebuf[:], mx[:], -1.0, lgs[:], ALU.mult, ALU.subtract,
            )
            nc.sync.dma_start(eout.ap(), ebuf[:])
            # weights: softmax over views of -E
            mn = stage.tile([D, NCHUNK], f32, tag="mn")
            ev = ebuf[:].rearrange("p (ch v) -> p ch v", v=V)
            nc.vector.tensor_reduce(mn[:], ev, mybir.AxisListType.X, ALU.min)
            ub = stage.tile([D, NCV], f32, tag="ub")
            mnb = mn[:].unsqueeze(2).broadcast_to([D, NCHUNK, V])
            nc.vector.scalar_tensor_tensor(
                ub[:].rearrange("p (ch v) -> p ch v", v=V),
                ev, -1.0, mnb, ALU.mult, ALU.add,
            )
            nc.scalar.activation(ub[:], ub[:], AF.Exp)
            su = stage.tile([D, NCHUNK], f32, tag="su")
            nc.vector.tensor_reduce(
                su[:], ub[:].rearrange("p (ch v) -> p ch v", v=V),
                mybir.AxisListType.X, ALU.add,
            )
            rcp = stage.tile([D, NCHUNK], f32, tag="rcp")
            nc.vector.reciprocal(rcp[:], su[:])
            wb = stage.tile([D, NCV], f32, tag="wb")
            rcb = rcp[:].unsqueeze(2).broadcast_to([D, NCHUNK, V])
            nc.vector.tensor_tensor(
                wb[:].rearrange("p (ch v) -> p ch v", v=V),
                ub[:].rearrange("p (ch v) -> p ch v", v=V),
                rcb, ALU.mult,
            )
            nc.sync.dma_start(wout.ap(), wb[:])

    _split_multi_waits(nc)
    return nc


def zTv(t, v):
    """Index a [V, ...] DRAM tensor ap down to view v."""
    return t.ap()[v]


def _tf32(x):
    """Round fp32 to fp32r (TF32: 10 explicit mantissa bits), RNE."""
    u = np.ascontiguousarray(x, dtype=np.float32).view(np.uint32)
    r = ((u >> 13) & np.uint32(1)).astype(np.uint32)
    u = (u + np.uint32(0x0FFF) + r) & np.uint32(0xFFFFE000)
    return u.view(np.float32)


def _host_params(phi, mu, sigma):
    """fp64 host-side factorization of the tiny GMM params."""
    S = sigma.astype(np.float64) + 1e-6 * np.eye(D)          # [V,K,D,D]
    Ls = np.linalg.cholesky(S)
    logdet = 2.0 * np.log(np.diagonal(Ls, axis1=-2, axis2=-1)).sum(-1)  # [V,K]
    A = np.linalg.inv(S)
    A = 0.5 * (A + A.transpose(0, 1, 3, 2))
    La = np.linalg.cholesky(A)                               # [V,K,D,D]
    b = np.einsum("vkde,vke->vkd", A, mu.astype(np.float64))  # [V,K,D]
    c = np.einsum("vkd,vkd->vk", mu.astype(np.float64), b)    # [V,K]
    ph = phi.astype(np.float64)
    logpi = ph - (np.max(ph, axis=1, keepdims=True)
                  + np.log(np.exp(ph - np.max(ph, axis=1, keepdims=True))
                           .sum(axis=1, keepdims=True)))
    C = -0.5 * c - 0.5 * D * LOG_2PI - 0.5 * logdet + logpi   # [V,K]

    Wstk = np.zeros((V, D, WCOLS), dtype=np.float64)
    for v in range(V):
        for k in range(K):
            Wstk[v, :, k * D:(k + 1) * D] = La[v, k]
            Wstk[v, :, K * D + k] = b[v, k]
    Wh = _tf32(Wstk.astype(np.float32))
    Wl = _tf32((Wstk - Wh.astype(np.float64)).astype(np.float32))
    Ctab = np.tile(C.reshape(1, V * K).astype(np.float32), (D, 1))

    # penalty is a pure function of sigma
    diag_S = np.diagonal(S, axis1=-2, axis2=-1)
    penalty = np.float32((1.0 / (diag_S + 1e-12)).sum())
    return Wh, Wl, Ctab, penalty


_NC_CACHE = None


def kernel(z, phi, mu, sigma):
    global LAST_RESULTS, _NC_CACHE
    z = np.asarray(z, dtype=np.float32)
    phi = np.asarray(phi, dtype=np.float32)
    mu = np.asarray(mu, dtype=np.float32)
    sigma = np.asarray(sigma, dtype=np.float32)

    Wh, Wl, Ctab, penalty = _host_params(phi, mu, sigma)

    if _NC_CACHE is None:
        _NC_CACHE = _build_program()
    nc = _NC_CACHE

    in_maps = []
    for core in range(NCORES):
        s = core * NPER
        zt = np.zeros((V, D, NPAD), dtype=np.float32)
        zt[:, :, :NPER] = z[:, s:s + NPER, :].transpose(0, 2, 1)
        zh = _tf32(zt)
        zl = _tf32(zt - zh)
        in_maps.append({"zTh": zh, "zTl": zl, "Wh": Wh, "Wl": Wl,
                        "Ctab": Ctab})

    res = run_bass_kernel_spmd(nc, in_maps, list(range(NCORES)))
    LAST_RESULTS = res

    energies = np.empty((N, V), dtype=np.float32)
    weights = np.empty((N, V), dtype=np.float32)
    for core in range(NCORES):
        s = core * NPER
        e = res.results[core]["eout"].reshape(D, NCHUNK, V)
        w = res.results[core]["wout"].reshape(D, NCHUNK, V)
        energies[s:s + NPER] = (
            e.transpose(1, 0, 2).reshape(NPAD, V)[:NPER]
        )
        weights[s:s + NPER] = (
            w.transpose(1, 0, 2).reshape(NPAD, V)[:NPER]
        )

    total_energies = np.float32(energies.astype(np.float64).sum())
    total_penalty = penalty
    return energies, weights, total_energies, total_penalty
